# revision 1
# baseline (speedup 1.0000x reference)
"""KNN-Attention Trainium2 kernel.

Sharding: 8 cores = 4 batches x 2 head-groups (8 heads each).
Each core computes a partial output [T, E] = combined_slice @ W_proj_slice;
host sums the two partials per batch.

Per-core device program (SPMD, all per-core variation flows through input data):
  - pass A: qT,kT [1024, T] bf16 matmuls (lhsT=W chunk, rhs=xT)
  - pass B: v natural [T, 512] bf16 (+ ones column for softmax denominators)
  - qnat:   q natural [T, 512] in TRUE fp32 (memory-attention scores need
            exact fp32 dots: softmax scale is E*sqrt(H)=4096, so tiny q errors
            flip near-tied slots vs the reference)
  - mem attention: DVE elementwise in natural layout (exact fp32 scores),
    softmax over K=3 slots, blend with g-prescaled mem_v; PE-transpose into
    combined^T
  - main attention per head: S^T tiles [128tk, 512tq] bf16, exp on ACT
    (scale 1/8, no max subtraction needed: |scores| <~ 3), causal masking via
    precomputed mask tiles on diagonal blocks, AV via lhsT=[v|1] producing
    O'^T [65, 512] with denominators in row 64
  - combine: combT += O'T * (1-g)/denom (partition-broadcast row)
  - c_proj: bf16 matmuls, PSUM -> DRAM partial output
"""

import numpy as np
import ml_dtypes

import concourse.bass as bass
import concourse.mybir as mybir
import concourse.tile as tile
from concourse import bacc
from concourse.bass_utils import run_bass_kernel_spmd
from concourse.masks import make_identity

B, T, E, H, KSLOT = 4, 2048, 1024, 16, 3
D = E // H          # 64
HPC = 8             # heads per core
EC = HPC * D        # 512 cols per core
NCORES = 8
TC = 512            # t-chunk
NCHUNK = T // TC    # 4

f32 = mybir.dt.float32
bf16 = mybir.dt.bfloat16

_CACHE = {}


def _build_nc():
    nc = bacc.Bacc("TRN2", target_bir_lowering=False, debug=False)

    # ---- DRAM I/O ----
    xTf = nc.dram_tensor("xTf", [E, T], f32, kind="ExternalInput").ap()
    xTb = nc.dram_tensor("xTb", [E, T], bf16, kind="ExternalInput").ap()
    wqkv = nc.dram_tensor("wqkv", [E, 3 * EC], bf16, kind="ExternalInput").ap()
    wq32 = nc.dram_tensor("wq32", [E, EC], f32, kind="ExternalInput").ap()
    wp = nc.dram_tensor("wp", [EC, E], bf16, kind="ExternalInput").ap()
    mk = nc.dram_tensor("mk", [T, KSLOT * EC], f32, kind="ExternalInput").ap()
    mvg = nc.dram_tensor("mvg", [T, KSLOT * EC], f32, kind="ExternalInput").ap()
    masks = nc.dram_tensor("masks", [4, 128, TC], bf16, kind="ExternalInput").ap()
    ginv = nc.dram_tensor("ginv", [1, HPC], f32, kind="ExternalInput").ap()
    out = nc.dram_tensor("out", [T, E], f32, kind="ExternalOutput").ap()

    # partition-tiled DRAM views
    xTf_r = xTf.rearrange("(ko p) t -> p ko t", p=128)      # [128, 8, T]
    xTb_r = xTb.rearrange("(ko p) t -> p ko t", p=128)
    wqkv_r = wqkv.rearrange("(ko p) n -> p ko n", p=128)    # [128, 8, 1536]
    wq32_r = wq32.rearrange("(ko p) n -> p ko n", p=128)    # [128, 8, 512]
    wp_r = wp.rearrange("(ko p) n -> p ko n", p=128)        # [128, 4, 1024]

    with tile.TileContext(nc) as tc:
        with (
            tc.tile_pool(name="consts", bufs=1) as consts,
            tc.tile_pool(name="chunk", bufs=2) as chunk,
            tc.tile_pool(name="xtf", bufs=1) as xtfp,
            tc.tile_pool(name="pt", bufs=4) as ptp,
            tc.tile_pool(name="small", bufs=2) as small,
            tc.tile_pool(name="pp", bufs=4, space="PSUM") as pp,
            tc.tile_pool(name="op", bufs=2, space="PSUM") as opp,
            tc.tile_pool(name="tp", bufs=2, space="PSUM") as tpp,
        ):
            # ---- persistent SBUF ----
            wqkv_sb = consts.tile([128, 8, 3 * EC], bf16, tag="wqkv")
            wq_sb = consts.tile([128, 8, EC], f32, tag="wq")
            wp_sb = consts.tile([128, 4, E], bf16, tag="wp")
            masks_sb = consts.tile([128, 4, TC], bf16, tag="masks")
            ginv_sb = consts.tile([1, HPC], f32, tag="ginv")
            ident = consts.tile([128, 128], f32, tag="ident")
            kT_sb = consts.tile([128, 4, T], bf16, tag="kT")
            v_sb = consts.tile([128, T // 128, HPC, D + 1], bf16, tag="v")

            nc.sync.dma_start(wqkv_sb[:], wqkv_r)
            nc.sync.dma_start(wq_sb[:], wq32_r)
            nc.sync.dma_start(wp_sb[:], wp_r)
            nc.sync.dma_start(masks_sb[:], masks.rearrange("m p t -> p m t"))
            nc.sync.dma_start(ginv_sb[:], ginv)
            make_identity(nc, ident[:])
            # ones column for softmax denominators
            nc.vector.memset(v_sb[:, :, :, D], 1.0)

            for c in range(NCHUNK):
                ts = slice(c * TC, (c + 1) * TC)
                xtb_c = chunk.tile([128, 8, TC], bf16, tag="xtb")
                nc.sync.dma_start(xtb_c[:], xTb_r[:, :, ts])
                xtf_c = xtfp.tile([128, 8, TC], f32, tag="xtf")
                nc.sync.dma_start(xtf_c[:], xTf_r[:, :, ts])

                qT_c = chunk.tile([128, 4, TC], bf16, tag="qT")
                combT_c = chunk.tile([128, 4, TC], bf16, tag="combT")

                # ---- pass A: qT (m 0-3) and kT (m 4-7), bf16 ----
                for m in range(8):
                    ps = pp.tile([128, TC], f32, tag="pp512")
                    for ke in range(8):
                        nc.tensor.matmul(
                            ps[:],
                            wqkv_sb[:, ke, 128 * m : 128 * (m + 1)],
                            xtb_c[:, ke, :],
                            start=(ke == 0),
                            stop=(ke == 7),
                        )
                    if m < 4:
                        nc.vector.tensor_copy(qT_c[:, m, :], ps[:])
                    else:
                        nc.vector.tensor_copy(kT_sb[:, m - 4, ts], ps[:])

                # ---- pass B: v natural [TC, 512] bf16 ----
                for tb in range(4):
                    ps = pp.tile([128, TC], f32, tag="pp512")
                    for ke in range(8):
                        nc.tensor.matmul(
                            ps[:],
                            xtb_c[:, ke, 128 * tb : 128 * (tb + 1)],
                            wqkv_sb[:, ke, 2 * EC : 3 * EC],
                            start=(ke == 0),
                            stop=(ke == 7),
                        )
                    for h in range(HPC):
                        nc.vector.tensor_copy(
                            v_sb[:, 4 * c + tb, h, 0:D], ps[:, D * h : D * (h + 1)]
                        )

                # ---- qnat (true fp32) + memory attention per t-block ----
                for tb in range(4):
                    trow = slice(c * TC + 128 * tb, c * TC + 128 * (tb + 1))
                    ps = pp.tile([128, TC], f32, tag="pp512")
                    for ke in range(8):
                        nc.tensor.matmul(
                            ps[:],
                            xtf_c[:, ke, 128 * tb : 128 * (tb + 1)],
                            wq_sb[:, ke, :],
                            start=(ke == 0),
                            stop=(ke == 7),
                        )
                    qn = chunk.tile([128, EC], f32, tag="qn")
                    nc.vector.tensor_copy(qn[:], ps[:])

                    mk_t = chunk.tile([128, KSLOT, EC], f32, tag="mk")
                    nc.sync.dma_start(
                        mk_t[:], mk[trow, :].rearrange("p (k e) -> p k e", k=KSLOT)
                    )
                    mv_t = chunk.tile([128, KSLOT, EC], f32, tag="mv")
                    nc.sync.dma_start(
                        mv_t[:], mvg[trow, :].rearrange("p (k e) -> p k e", k=KSLOT)
                    )

                    # exact fp32 scores: s3[t, k, h] = sum_d qn*mk
                    prod = chunk.tile([128, KSLOT, EC], f32, tag="prod")
                    nc.vector.tensor_mul(
                        prod[:], mk_t[:], qn[:, None, :].to_broadcast((128, KSLOT, EC))
                    )
                    s3 = small.tile([128, KSLOT, HPC], f32, tag="s3")
                    nc.vector.tensor_reduce(
                        s3[:],
                        prod[:].rearrange("p k (h d) -> p k h d", d=D),
                        mybir.AxisListType.X,
                        mybir.AluOpType.add,
                    )
                    m3 = small.tile([128, HPC], f32, tag="m3")
                    nc.vector.tensor_reduce(
                        m3[:],
                        s3[:].rearrange("p k h -> p h k"),
                        mybir.AxisListType.X,
                        mybir.AluOpType.max,
                    )
                    z3 = small.tile([128, KSLOT, HPC], f32, tag="z3")
                    nc.vector.tensor_sub(
                        z3[:], s3[:], m3[:, None, :].to_broadcast((128, KSLOT, HPC))
                    )
                    e3 = small.tile([128, KSLOT, HPC], f32, tag="e3")
                    nc.scalar.activation(
                        e3[:], z3[:], mybir.ActivationFunctionType.Exp,
                        scale=float(E) * float(np.sqrt(H)),
                    )
                    den = small.tile([128, HPC], f32, tag="den")
                    nc.vector.tensor_reduce(
                        den[:],
                        e3[:].rearrange("p k h -> p h k"),
                        mybir.AxisListType.X,
                        mybir.AluOpType.add,
                    )
                    rden = small.tile([128, HPC], f32, tag="rden")
                    nc.vector.reciprocal(rden[:], den[:])
                    w3 = small.tile([128, KSLOT, HPC], f32, tag="w3")
                    nc.vector.tensor_mul(
                        w3[:], e3[:], rden[:, None, :].to_broadcast((128, KSLOT, HPC))
                    )
                    # blend: mm[t, e] = sum_k w3[t,k,h(e)] * mvg[t,k,e]
                    wprod = chunk.tile([128, KSLOT, EC], f32, tag="prod")
                    nc.vector.tensor_mul(
                        wprod[:].rearrange("p k (h d) -> p k h d", d=D),
                        mv_t[:].rearrange("p k (h d) -> p k h d", d=D),
                        w3[:, :, :, None].to_broadcast((128, KSLOT, HPC, D)),
                    )
                    mm_t = chunk.tile([128, EC], f32, tag="mm")
                    nc.vector.tensor_add(mm_t[:], wprod[:, 0, :], wprod[:, 1, :])
                    nc.vector.tensor_add(mm_t[:], mm_t[:], wprod[:, 2, :])

                    # transpose mem output into combT (combT = g*mem part; g
                    # was folded into mvg on host)
                    for ec in range(4):
                        tps = tpp.tile([128, 128], f32, tag="tp")
                        nc.tensor.transpose(
                            tps[:], mm_t[:, 128 * ec : 128 * (ec + 1)], ident[:]
                        )
                        nc.scalar.activation(
                            combT_c[:, ec, 128 * tb : 128 * (tb + 1)], tps[:],
                            mybir.ActivationFunctionType.Copy,
                        )

                # ---- main attention per head ----
                for h in range(HPC):
                    prow = slice(64 * (h % 2), 64 * (h % 2) + 64)
                    pc = h // 2
                    ops = opp.tile([65, TC], f32, tag="ops")
                    njt = 4 * c + 4
                    for j in range(njt):
                        sps = pp.tile([128, TC], f32, tag="pp512")
                        nc.tensor.matmul(
                            sps[:],
                            kT_sb[prow, pc, 128 * j : 128 * (j + 1)],
                            qT_c[prow, pc, :],
                            start=True,
                            stop=True,
                        )
                        pt = ptp.tile([128, TC], bf16, tag="pt")
                        nc.scalar.activation(
                            pt[:], sps[:], mybir.ActivationFunctionType.Exp,
                            scale=1.0 / np.sqrt(D),
                        )
                        if j >= 4 * c:
                            nc.vector.tensor_mul(
                                pt[:], pt[:], masks_sb[:, j - 4 * c, :]
                            )
                        nc.tensor.matmul(
                            ops[:],
                            v_sb[:, j, h, :],
                            pt[:],
                            start=(j == 0),
                            stop=(j == njt - 1),
                        )
                    # normalize + accumulate into combT
                    rr = small.tile([1, TC], f32, tag="rr")
                    nc.vector.reciprocal(rr[:], ops[64:65, :])
                    rr2 = small.tile([1, TC], f32, tag="rr2")
                    nc.vector.tensor_scalar_mul(rr2[:], rr[:], ginv_sb[0:1, h : h + 1])
                    bc = small.tile([128, TC], f32, tag="bc")
                    nc.gpsimd.partition_broadcast(bc[:], rr2[0:1, :])
                    tmp = small.tile([128, TC], f32, tag="tmpo")
                    nc.vector.tensor_mul(tmp[prow, :], ops[0:64, :], bc[prow, :])
                    nc.vector.tensor_add(
                        combT_c[prow, pc, :], combT_c[prow, pc, :], tmp[prow, :]
                    )

                # ---- c_proj partial: out[tc-rows, :] ----
                for tb in range(4):
                    trow = slice(c * TC + 128 * tb, c * TC + 128 * (tb + 1))
                    for n in range(2):
                        ps = pp.tile([128, TC], f32, tag="pp512")
                        for ke in range(4):
                            nc.tensor.matmul(
                                ps[:],
                                combT_c[:, ke, 128 * tb : 128 * (tb + 1)],
                                wp_sb[:, ke, TC * n : TC * (n + 1)],
                                start=(ke == 0),
                                stop=(ke == 3),
                            )
                        ost = chunk.tile([128, TC], f32, tag="ost")
                        nc.vector.tensor_copy(ost[:], ps[:])
                        nc.sync.dma_start(out[trow, TC * n : TC * (n + 1)], ost[:])

    nc.compile()
    return nc


def _prep_inputs(x, mem_k, mem_v, W_attn, W_proj, gate_bias):
    """Build per-core input maps (host-side sharding/layout only)."""
    in_maps = []
    g = gate_bias.reshape(H)
    for core in range(NCORES):
        b, hg = core // 2, core % 2
        cs = slice(hg * EC, (hg + 1) * EC)
        xb = np.asarray(x[b], dtype=np.float32)            # [T, E]
        xT = np.ascontiguousarray(xb.T)                    # [E, T]
        wq = np.ascontiguousarray(W_attn[:, cs])           # [E, 512]
        wk = np.ascontiguousarray(W_attn[:, E + hg * EC : E + (hg + 1) * EC])
        wv = np.ascontiguousarray(W_attn[:, 2 * E + hg * EC : 2 * E + (hg + 1) * EC])
        wqkv = np.concatenate([wq, wk, wv], axis=1)        # [E, 1536]
        gh = g[hg * HPC : (hg + 1) * HPC].astype(np.float32)   # [8]
        mkc = np.ascontiguousarray(mem_k[b][:, :, cs]).reshape(T, KSLOT * EC)
        mvc = np.ascontiguousarray(mem_v[b][:, :, cs]).astype(np.float32)
        # fold gate into mem_v: combined = mem*g + y*(1-g)
        mvc = mvc * gh.repeat(D)[None, None, :]
        mvc = mvc.reshape(T, KSLOT * EC)
        wpc = np.ascontiguousarray(W_proj[cs, :])          # [512, E]
        # causal mask tiles for the 4 diagonal sub-blocks
        m4 = np.zeros((4, 128, TC), dtype=np.float32)
        tkh = np.arange(128)[:, None]
        tqh = np.arange(TC)[None, :]
        for mi in range(4):
            m4[mi] = (128 * mi + tkh <= tqh).astype(np.float32)
        in_maps.append(
            {
                "xTf": xT,
                "xTb": xT.astype(ml_dtypes.bfloat16),
                "wqkv": wqkv.astype(ml_dtypes.bfloat16),
                "wq32": wq,
                "wp": wpc.astype(ml_dtypes.bfloat16),
                "mk": mkc.astype(np.float32),
                "mvg": mvc.astype(np.float32),
                "masks": m4.astype(ml_dtypes.bfloat16),
                "ginv": (1.0 - gh).reshape(1, HPC),
            }
        )
    return in_maps


def kernel(x, mem_k, mem_v, W_attn, W_proj, gate_bias, **kw):
    x = np.asarray(x, dtype=np.float32)
    mem_k = np.asarray(mem_k, dtype=np.float32)
    mem_v = np.asarray(mem_v, dtype=np.float32)
    W_attn = np.asarray(W_attn, dtype=np.float32)
    W_proj = np.asarray(W_proj, dtype=np.float32)
    gate_bias = np.asarray(gate_bias, dtype=np.float32)

    if "nc" not in _CACHE:
        _CACHE["nc"] = _build_nc()
    nc = _CACHE["nc"]
    in_maps = _prep_inputs(x, mem_k, mem_v, W_attn, W_proj, gate_bias)
    res = run_bass_kernel_spmd(nc, in_maps, list(range(NCORES)), **kw)
    results = res.results if hasattr(res, "results") else res
    out = np.empty((B, T, E), dtype=np.float32)
    for b in range(B):
        out[b] = results[2 * b]["out"] + results[2 * b + 1]["out"]
    _CACHE["last_res"] = res
    return out



# revision 66
# speedup vs baseline: 1.4296x; 1.4296x over previous
"""KNN-Attention Trainium2 kernel, v2.

Sharding: 8 cores = 4 batches x 2 head-groups (8 heads each).
Each core computes a partial output [T, E] = combined_slice @ W_proj_slice;
host sums the two partials per batch.

v2 changes vs baseline:
  - q via 3-term bf16 hi/lo (xh@Wh + xl@Wh + xh@Wl): fp32-grade memory
    scores at bf16 matmul rates; x ships as a bf16 hi/lo pair (no fp32 x)
  - qT and combT via SBUF->SBUF XBAR DMA transposes (off the PE)
  - attention AV in natural orientation: out[tq,65] = pt_blk^T @ [v|1],
    so softmax denominators land per-partition -> cheap fused
    scalar_tensor_tensor normalize, no partition_broadcast
  - exact causal column-trim on diagonal S tiles, exp over paired
    j-tiles, [128,128] triangle mask muls
  - one start=True per PSUM accumulation bank (start clears has_written
    for the whole bank; per-element overwrite covers later regions)
  - mem-attention in natural layout fused into comb; blend split across
    DVE/Pool; gate folded on host (g into mem_v bf16, 1-g into W_v)
  - software-pipelined schedule: chunk c attention overlaps chunk c+1
    projections/mem work; attention 2/3 overlap; c_proj deferred
"""

import numpy as np
import ml_dtypes

import concourse.bass as bass
import concourse.mybir as mybir
import concourse.tile as tile
from concourse import bacc
from concourse.bass_utils import run_bass_kernel_spmd
from concourse.masks import make_identity

B, T, E, H, KSLOT = 4, 2048, 1024, 16, 3
D = E // H          # 64
HPC = 8             # heads per core
EC = HPC * D        # 512 cols per core
NCORES = 8
TC = 512            # t-chunk
NCHUNK = T // TC    # 4

f32 = mybir.dt.float32
f32r = mybir.dt.float32r
bf16 = mybir.dt.bfloat16

_CACHE = {}


def _build_nc():
    nc = bacc.Bacc("TRN2", target_bir_lowering=False, debug=False)

    xTh = nc.dram_tensor("xTh", [E, T], bf16, kind="ExternalInput").ap()
    xTl = nc.dram_tensor("xTl", [E, T], bf16, kind="ExternalInput").ap()
    wkv = nc.dram_tensor("wkv", [E, 2 * EC], bf16, kind="ExternalInput").ap()
    wq2 = nc.dram_tensor("wq2", [E, 2 * EC], bf16, kind="ExternalInput").ap()
    wp = nc.dram_tensor("wp", [EC, E], bf16, kind="ExternalInput").ap()
    mk = nc.dram_tensor("mk", [T, KSLOT * EC], f32, kind="ExternalInput").ap()
    mvg = nc.dram_tensor("mvg", [T, KSLOT * EC], bf16, kind="ExternalInput").ap()
    masks = nc.dram_tensor("masks", [4, 128, TC], bf16, kind="ExternalInput").ap()
    out = nc.dram_tensor("out", [T, E], f32, kind="ExternalOutput").ap()

    xTh_r = xTh.rearrange("(ko p) t -> p ko t", p=128)      # [128, 8, T]
    xTl_r = xTl.rearrange("(ko p) t -> p ko t", p=128)
    wkv_r = wkv.rearrange("(ko p) n -> p ko n", p=128)      # [128, 8, 1024]
    wq2_r = wq2.rearrange("(ko p) n -> p ko n", p=128)      # [128, 8, 1024] (wqh|wql)
    wp_r = wp.rearrange("(ko p) n -> p ko n", p=128)        # [128, 4, 1024]

    with tile.TileContext(nc) as tc:
        with (
            tc.tile_pool(name="consts", bufs=1) as consts,
            tc.tile_pool(name="xtf", bufs=2) as xtfp,
            tc.tile_pool(name="chunk", bufs=4) as chunk,
            tc.tile_pool(name="mem", bufs=2) as memp,
            tc.tile_pool(name="pt", bufs=4) as ptp,
            tc.tile_pool(name="small", bufs=3) as small,
            tc.tile_pool(name="pA", bufs=2, space="PSUM") as pA,
            tc.tile_pool(name="pS", bufs=2, space="PSUM") as pS,
            tc.tile_pool(name="pO", bufs=2, space="PSUM") as pO,
        ):
            # ---- persistent SBUF ----
            wkv_sb = consts.tile([128, 8, 2 * EC], bf16, tag="wkv")
            wq_sb = consts.tile([128, 8, 2 * EC], bf16, tag="wq")
            wp_sb = consts.tile([128, 4, E], bf16, tag="wp")
            masks_sb = consts.tile([128, 4, TC], bf16, tag="masks")
            kT_sb = consts.tile([128, 4, T], bf16, tag="kT")
            v_sb = consts.tile([128, T // 128, HPC, D + 1], bf16, tag="v")

            nc.vector.memset(v_sb[:, :, :, D], 1.0)
            # chunk-0 x first so PE can start right after slice-0 weights land
            xtb0 = xtfp.tile([128, 8, TC], bf16, tag="xtb")
            nc.sync.dma_start(xtb0[:, 0:4, :], xTh_r[:, 0:4, 0:TC])
            nc.sync.dma_start(xtb0[:, 4:8, :], xTh_r[:, 4:8, 0:TC])
            for ke in range(8):
                nc.sync.dma_start(wkv_sb[:, ke, :], wkv_r[:, ke, :])
            xlo0 = xtfp.tile([128, 8, TC], bf16, tag="xlo")
            nc.sync.dma_start(xlo0[:], xTl_r[:, :, 0:TC])
            for ke in range(8):
                nc.sync.dma_start(wq_sb[:, ke, :], wq2_r[:, ke, :])
            nc.sync.dma_start(masks_sb[:], masks.rearrange("m p t -> p m t"))
            nc.sync.dma_start(wp_sb[:], wp_r)

            cs = {}  # chunk state: (qT_c, comb_c)

            def gemm_units(c, x_pre=None):
                """kT/v/q projections + mem attention for chunk c."""
                ts = slice(c * TC, (c + 1) * TC)
                if x_pre is None:
                    xtb_c = xtfp.tile([128, 8, TC], bf16, tag="xtb")
                    nc.sync.dma_start(xtb_c[:], xTh_r[:, :, ts])
                    xlo_c = xtfp.tile([128, 8, TC], bf16, tag="xlo")
                    nc.sync.dma_start(xlo_c[:], xTl_r[:, :, ts])
                else:
                    xtb_c, xlo_c = x_pre
                qT_c = chunk.tile([128, 4, TC], bf16, tag="qT")
                comb_c = chunk.tile([128, 4, TC], f32, tag="comb", bufs=3)
                cs[c] = (qT_c, comb_c)

                for m in range(4):  # kT[d, t] = sum_e Wk[e,d] xT[e,t]
                    ps = pA.tile([128, TC], f32, tag="pA")
                    for ke in range(8):
                        nc.tensor.matmul(
                            ps[:],
                            wkv_sb[:, ke, 128 * m : 128 * (m + 1)],
                            xtb_c[:, ke, :],
                            start=(ke == 0), stop=(ke == 7),
                        )
                    nc.vector.tensor_copy(kT_sb[:, m, ts], ps[:])
                    yield

                for tb in range(4):
                    trow = slice(c * TC + 128 * tb, c * TC + 128 * (tb + 1))
                    # v natural [128t, 512]
                    ps = pA.tile([128, TC], f32, tag="pA")
                    for ke in range(8):
                        nc.tensor.matmul(
                            ps[:],
                            xtb_c[:, ke, 128 * tb : 128 * (tb + 1)],
                            wkv_sb[:, ke, EC : 2 * EC],
                            start=(ke == 0), stop=(ke == 7),
                        )
                    nc.scalar.copy(
                        v_sb[:, 4 * c + tb, :, 0:D],
                        ps[:].rearrange("p (h d) -> p h d", d=D),
                    )
                    # q natural [128t, 512]: exact-ish via 3-term bf16 hi/lo
                    qps = pA.tile([128, TC], f32, tag="pA")
                    for ke in range(8):
                        nc.tensor.matmul(
                            qps[:],
                            xtb_c[:, ke, 128 * tb : 128 * (tb + 1)],
                            wq_sb[:, ke, 0:EC],
                            start=(ke == 0), stop=False,
                        )
                    for ke in range(8):
                        nc.tensor.matmul(
                            qps[:],
                            xlo_c[:, ke, 128 * tb : 128 * (tb + 1)],
                            wq_sb[:, ke, 0:EC],
                            start=False, stop=False,
                        )
                    for ke in range(8):
                        nc.tensor.matmul(
                            qps[:],
                            xtb_c[:, ke, 128 * tb : 128 * (tb + 1)],
                            wq_sb[:, ke, EC : 2 * EC],
                            start=False, stop=(ke == 7),
                        )
                    qnb = small.tile([128, TC], bf16, tag="qnb", bufs=2)
                    nc.vector.tensor_copy(qnb[:], qps[:])
                    qn = small.tile([128, TC], f32, tag="qn")
                    nc.scalar.copy(qn[:], qps[:])
                    nc.sync.dma_start_transpose(
                        qT_c[:, :, 128 * tb : 128 * (tb + 1)], qnb[:]
                    )

                    # mem attention (natural): exact fp32 scores vs PSUM q
                    mk_t = memp.tile([128, KSLOT, EC], f32, tag="mk", bufs=3)
                    nc.sync.dma_start(
                        mk_t[:], mk[trow, :].rearrange("p (k e) -> p k e", k=KSLOT)
                    )
                    mv_t = memp.tile([128, KSLOT, EC], bf16, tag="mv")
                    nc.sync.dma_start(
                        mv_t[:], mvg[trow, :].rearrange("p (k e) -> p k e", k=KSLOT)
                    )
                    nc.vector.tensor_mul(
                        mk_t[:], mk_t[:],
                        qn[:, None, :].to_broadcast((128, KSLOT, EC)),
                    )
                    s3 = small.tile([128, KSLOT, HPC], f32, tag="s3")
                    nc.vector.tensor_reduce(
                        s3[:],
                        mk_t[:].rearrange("p k (h d) -> p k h d", d=D),
                        mybir.AxisListType.X, mybir.AluOpType.add,
                    )
                    m3 = small.tile([128, HPC], f32, tag="m3")
                    nc.vector.tensor_reduce(
                        m3[:], s3[:].rearrange("p k h -> p h k"),
                        mybir.AxisListType.X, mybir.AluOpType.max,
                    )
                    z3 = small.tile([128, KSLOT, HPC], f32, tag="z3")
                    nc.vector.tensor_sub(
                        z3[:], s3[:], m3[:, None, :].to_broadcast((128, KSLOT, HPC))
                    )
                    e3 = small.tile([128, KSLOT, HPC], f32, tag="e3")
                    nc.scalar.activation(
                        e3[:], z3[:], mybir.ActivationFunctionType.Exp,
                        scale=float(E) * float(np.sqrt(H)),
                    )
                    den3 = small.tile([128, HPC], f32, tag="den3")
                    nc.vector.tensor_reduce(
                        den3[:], e3[:].rearrange("p k h -> p h k"),
                        mybir.AxisListType.X, mybir.AluOpType.add,
                    )
                    rden3 = small.tile([128, HPC], f32, tag="rden3")
                    nc.vector.reciprocal(rden3[:], den3[:])
                    w3 = small.tile([128, KSLOT, HPC], f32, tag="w3")
                    nc.vector.tensor_mul(
                        w3[:], e3[:],
                        rden3[:, None, :].to_broadcast((128, KSLOT, HPC)),
                    )
                    wprod = memp.tile([128, KSLOT, EC], bf16, tag="wprod")
                    weng = nc.gpsimd if tb % 2 == 0 else nc.vector
                    weng.tensor_mul(
                        wprod[:].rearrange("p k (h d) -> p k h d", d=D),
                        mv_t[:].rearrange("p k (h d) -> p k h d", d=D),
                        w3[:, :, :, None].to_broadcast((128, KSLOT, HPC, D)),
                    )
                    cb = comb_c[:, tb, :]
                    aeng = nc.vector if tb % 2 == 0 else nc.gpsimd
                    aeng.tensor_add(cb, wprod[:, 0, :], wprod[:, 1, :])
                    nc.gpsimd.tensor_add(cb, cb, wprod[:, 2, :])

                    yield

            def attn_units(c):
                """Main attention + combine for chunk c."""
                qT_c, comb_c = cs[c]

                for h in range(HPC):
                    prow = slice(64 * (h % 2), 64 * (h % 2) + 64)
                    pc = h // 2
                    onat = pO.tile([128, 4, D + 1], f32, tag="onat")

                    items = []
                    for jj in range(0, 4 * c, 2):
                        items.append(([jj, jj + 1], [0, 0]))
                    items.append(([4 * c, 4 * c + 1], [0, 128]))
                    items.append(([4 * c + 2, 4 * c + 3], [256, 384]))

                    def issue_S(item):
                        jl, lo = item
                        sp = pS.tile([128, 2, TC], f32, tag="sp")
                        pt = ptp.tile([128, 2, TC], bf16, tag="pt")
                        for i, (j, l) in enumerate(zip(jl, lo)):
                            nc.tensor.matmul(
                                sp[:, i, l:TC],
                                kT_sb[prow, pc, 128 * j : 128 * (j + 1)],
                                qT_c[prow, pc, l:TC],
                                start=True, stop=True,
                            )
                        lo0 = min(lo)
                        nc.scalar.activation(
                            pt[:, 0:2, lo0:TC], sp[:, 0:2, lo0:TC],
                            mybir.ActivationFunctionType.Exp,
                            scale=1.0 / float(np.sqrt(D)),
                        )
                        if jl[0] >= 4 * c:  # diagonal pair: block triangle masks
                            for i, (j, l) in enumerate(zip(jl, lo)):
                                m = j - 4 * c
                                nc.vector.tensor_mul(
                                    pt[:, i, l : l + 128], pt[:, i, l : l + 128],
                                    masks_sb[:, m, l : l + 128],
                                )
                        return (jl, lo, pt)

                    def issue_AV(st):
                        # start=True clears has_written for the WHOLE bank, so
                        # only the very first matmul into this onat tile may
                        # set it; later first-writes of other regions rely on
                        # per-element overwrite-where-bit-unset.
                        jl, lo, pt = st
                        for i, (j, l) in enumerate(zip(jl, lo)):
                            for tqb in range(l // 128, 4):
                                nc.tensor.matmul(
                                    onat[:, tqb, :],
                                    pt[:, i, 128 * tqb : 128 * (tqb + 1)],
                                    v_sb[:, j, h, :],
                                    start=(j == 0 and tqb == 0),
                                    stop=(j == 4 * c + tqb),
                                    skip_group_check=True,
                                )

                    prev = issue_S(items[0])
                    yield
                    for it in items[1:]:
                        cur = issue_S(it)
                        issue_AV(prev)
                        prev = cur
                        yield
                    issue_AV(prev)

                    rdy = small.tile([128, 4], f32, tag="rdy")
                    nc.vector.reciprocal(rdy[:], onat[:, :, D])
                    for tqb in range(4):
                        cslice = comb_c[:, tqb, D * h : D * (h + 1)]
                        nc.vector.scalar_tensor_tensor(
                            cslice,
                            onat[:, tqb, 0:D],
                            rdy[:, tqb : tqb + 1],
                            cslice,
                            mybir.AluOpType.mult,
                            mybir.AluOpType.add,
                        )
                    yield

            def cproj_units(c):
                qT_c, comb_c = cs[c]
                for tb in range(4):
                    trow = slice(c * TC + 128 * tb, c * TC + 128 * (tb + 1))
                    combb = small.tile([128, TC], bf16, tag="combb")
                    nc.vector.tensor_copy(combb[:], comb_c[:, tb, :])
                    combT = small.tile([128, 4, 128], bf16, tag="combT")
                    nc.sync.dma_start_transpose(combT[:], combb[:])
                    for n in range(2):
                        ps = pA.tile([128, TC], f32, tag="pA")
                        for ke in range(4):
                            nc.tensor.matmul(
                                ps[:],
                                combT[:, ke, :],
                                wp_sb[:, ke, TC * n : TC * (n + 1)],
                                start=(ke == 0), stop=(ke == 3),
                            )
                        ost = small.tile([128, TC], f32, tag="ost")
                        nc.vector.tensor_copy(ost[:], ps[:])
                        nc.sync.dma_start(out[trow, TC * n : TC * (n + 1)], ost[:])
                    yield

            # ---- schedule: pairwise overlap, c_proj deferred to tail ----
            def drain(it):
                for _ in it:
                    pass

            def chain(*its):
                for it in its:
                    yield from it

            def interleave(a, g, ratio):
                """ratio units of a per 1 of g until a ends; returns g."""
                done_a = done_g = False
                while not done_a:
                    for _ in range(ratio):
                        if not done_a:
                            try:
                                next(a)
                            except StopIteration:
                                done_a = True
                    if not done_g:
                        try:
                            next(g)
                        except StopIteration:
                            done_g = True
                return None if done_g else g

            drain(gemm_units(0, x_pre=(xtb0, xlo0)))
            l1 = interleave(attn_units(0), gemm_units(1), 2)
            aux2 = chain(l1, gemm_units(2), cproj_units(0)) if l1 is not None \
                else chain(gemm_units(2), cproj_units(0))
            l2 = interleave(attn_units(1), aux2, 2)
            a2 = attn_units(2)
            g3 = chain(l2, gemm_units(3), cproj_units(1)) if l2 is not None \
                else chain(gemm_units(3), cproj_units(1))
            a3 = None
            done_g3 = False
            a2_done = False
            n_g3 = 0
            while not done_g3:
                try:
                    next(a2)
                    next(a2)
                except StopIteration:
                    a2_done = True
                try:
                    next(g3)
                    n_g3 += 1
                except StopIteration:
                    done_g3 = True
                # once gemm(3) is fully issued (12 units), start feeding a3
                if n_g3 >= 13 and a3 is None:
                    a3 = attn_units(3)
                if a3 is not None:
                    try:
                        next(a3)
                        next(a3)
                    except StopIteration:
                        pass
            if a3 is None:
                a3 = attn_units(3)
            if not a2_done:
                rest_a3 = interleave(a2, a3, 1)
            else:
                rest_a3 = a3
            p2 = cproj_units(2)
            if rest_a3 is not None:
                p2 = interleave(rest_a3, p2, 3)
            p3 = cproj_units(3)
            done2 = p2 is None
            done3 = False
            while not (done2 and done3):
                if not done2:
                    try:
                        next(p2)
                    except StopIteration:
                        done2 = True
                if not done3:
                    try:
                        next(p3)
                    except StopIteration:
                        done3 = True

    nc.compile()
    return nc


def _prep_inputs(x, mem_k, mem_v, W_attn, W_proj, gate_bias):
    """Build per-core input maps (host-side sharding/layout only)."""
    in_maps = []
    g = gate_bias.reshape(H)
    m4 = np.zeros((4, 128, TC), dtype=np.float32)
    tkh = np.arange(128)[:, None]
    tqh = np.arange(TC)[None, :]
    for mi in range(4):
        m4[mi] = (128 * mi + tkh <= tqh).astype(np.float32)
    m4 = m4.astype(ml_dtypes.bfloat16)
    for core in range(NCORES):
        b, hg = core // 2, core % 2
        cs = slice(hg * EC, (hg + 1) * EC)
        gh = g[hg * HPC : (hg + 1) * HPC].astype(np.float32)       # [8]
        xb = np.asarray(x[b], dtype=np.float32)                    # [T, E]
        xT = np.ascontiguousarray(xb.T)                            # [E, T]
        xTh_ = xT.astype(ml_dtypes.bfloat16)
        xTl_ = (xT - xTh_.astype(np.float32)).astype(ml_dtypes.bfloat16)
        wq = np.ascontiguousarray(W_attn[:, cs])                   # [E, 512]
        wqh_ = wq.astype(ml_dtypes.bfloat16)
        wql_ = (wq - wqh_.astype(np.float32)).astype(ml_dtypes.bfloat16)
        wq2_ = np.concatenate([wqh_, wql_], axis=1)                # [E, 1024]
        wk = np.ascontiguousarray(W_attn[:, E + hg * EC : E + (hg + 1) * EC])
        wv = np.ascontiguousarray(
            W_attn[:, 2 * E + hg * EC : 2 * E + (hg + 1) * EC]
        ).copy()
        # fold (1-g) into Wv columns (per-head 64-col blocks)
        wv *= (1.0 - gh).repeat(D)[None, :]
        wkv = np.concatenate([wk, wv], axis=1)                     # [E, 1024]
        mkc = np.ascontiguousarray(mem_k[b][:, :, cs]).reshape(T, KSLOT * EC)
        mvc = np.ascontiguousarray(mem_v[b][:, :, cs]).astype(np.float32)
        mvc = mvc * gh.repeat(D)[None, None, :]                    # fold g
        mvc = mvc.reshape(T, KSLOT * EC)
        wpc = np.ascontiguousarray(W_proj[cs, :])                  # [512, E]
        in_maps.append(
            {
                "xTh": xTh_,
                "xTl": xTl_,
                "wkv": wkv.astype(ml_dtypes.bfloat16),
                "wq2": wq2_,
                "wp": wpc.astype(ml_dtypes.bfloat16),
                "mk": mkc.astype(np.float32),
                "mvg": mvc.astype(ml_dtypes.bfloat16),
                "masks": m4,
            }
        )
    return in_maps


def kernel(x, mem_k, mem_v, W_attn, W_proj, gate_bias, **kw):
    x = np.asarray(x, dtype=np.float32)
    mem_k = np.asarray(mem_k, dtype=np.float32)
    mem_v = np.asarray(mem_v, dtype=np.float32)
    W_attn = np.asarray(W_attn, dtype=np.float32)
    W_proj = np.asarray(W_proj, dtype=np.float32)
    gate_bias = np.asarray(gate_bias, dtype=np.float32)

    if "nc" not in _CACHE:
        _CACHE["nc"] = _build_nc()
    nc = _CACHE["nc"]
    in_maps = _prep_inputs(x, mem_k, mem_v, W_attn, W_proj, gate_bias)
    res = run_bass_kernel_spmd(nc, in_maps, list(range(NCORES)), **kw)
    results = res.results if hasattr(res, "results") else res
    out = np.empty((B, T, E), dtype=np.float32)
    for b in range(B):
        out[b] = results[2 * b]["out"] + results[2 * b + 1]["out"]
    _CACHE["last_res"] = res
    return out
